# revision 1
# baseline (speedup 1.0000x reference)
"""DenseEnergyLoss on 8 Trainium2 NeuronCores (Bass/Tile).

Reference computes, per image: a [P,P] Gaussian bilateral affinity
Wm = exp(-0.5*d2(f_p,f_q)) over 5-dim features f = (x/sxy, y/sxy, rgb/15),
then loss = -W/N * sum(S * ((S @ Wm) * gate)) with S = seg_roi, P = 64*64.

Device formulation (v2, symmetric-triangle):
  Wm is symmetric, so  sum_{p,q} S_p Wm SG_q
    = sum_{p<q} (S_p SG_q + SG_p S_q) Wm_pq + sum_p S_p SG_p Wm_pp.
  Each core owns 16 of the 32 p-blocks of one image (parity-interleaved:
  core parity h takes global blocks g = 2i+h) and computes only tiles
  (block i, q-chunk c) on/above the diagonal. Both pair-orientations are
  produced by ONE matmul with the stacked stationary operand [S_i | SG_i]
  (K=128 -> M=42). Diagonal-band tiles are masked elementwise with a
  host-built {0,1,0.5} mask (4 variants per chunk; parity baked into the
  mask DATA so the program is identical across cores).

  exponent X[p,q] = u_p . v_q with u = [f, -0.5|f|^2, 1], v = [f, 1, -0.5|f|^2]
  computed as a K=21 fp16 matmul with an error-compensated hi/lo split
  stacked along the contraction dim (exact products, fp32 PSUM accumulate).
  Wm = exp(X) on the scalar engine (fp16 out; fp16 rounding absorbs the
  reference's d2>=0 clamp). Final loss partial = sum(AS * [SG;S]) via
  vector-engine multiply + free-dim reduction.

Host does only the cheap O(P) prep: stride-2 subsample (nearest resize),
2x2 avg pool (bilinear resize at scale 0.5), gating, feature build, masks,
and the final sum of the 8 per-core partials.
"""

import numpy as np

# problem shapes (hardcoded per contract)
N_IMG = 4
K = 21
K2 = 2 * K             # stacked [S|SG] output rows
H = 128
W = 128
HO, WO = 64, 64
P = HO * WO            # 4096
HALF = P // 2          # p-rows per core
NBLK = HALF // 128     # 16 local p-blocks of 128 per core
QCH = 1024             # q-chunk width (2 PSUM banks)
NCH = P // QCH
N_CORES = 8
KF = 21                # contraction dim of the feature matmul (3*7)
NMASK = 4              # band-mask variants per chunk
MW = 256               # mask width: band block m covers chunk cols [MW*m, QCH)

SIGMA_RGB = 15.0
SXY = 100.0 * 0.5      # SIGMA_XY * SCALE
WEIGHT = 1e-7

_CACHE = {}


def _build_module(loop_n=1):
    from contextlib import ExitStack

    import concourse.bacc as bacc
    import concourse.tile as tile
    from concourse import mybir

    fp32 = mybir.dt.float32
    fp16 = mybir.dt.float16

    nc = bacc.Bacc(trn_type="TRN2", target_bir_lowering=False, debug=False)

    UH = nc.declare_dram_parameter("UH", [KF, HALF], fp16, isOutput=False)
    VH = nc.declare_dram_parameter("VH", [KF, P], fp16, isOutput=False)
    STG = nc.declare_dram_parameter("STG", [128, NBLK * K2], fp16, isOutput=False)
    SGS = nc.declare_dram_parameter("SGS", [K2, P], fp32, isOutput=False)
    MASKS = nc.declare_dram_parameter("MASKS", [128, NMASK * MW], fp16,
                                      isOutput=False)
    OUT = nc.declare_dram_parameter("out", [K2, 1], fp32, isOutput=True)

    with tile.TileContext(nc) as tc, ExitStack() as ctx:
        singles = ctx.enter_context(tc.tile_pool(name="singles", bufs=1))
        gpool = ctx.enter_context(tc.tile_pool(name="g", bufs=2, space="PSUM"))
        aspool = ctx.enter_context(tc.tile_pool(name="as", bufs=2, space="PSUM"))
        wpool = ctx.enter_context(tc.tile_pool(name="wm", bufs=3))
        epool = ctx.enter_context(tc.tile_pool(name="evac", bufs=2))
        accp = ctx.enter_context(tc.tile_pool(name="acc", bufs=2))

        # DMAs ordered by first use; VH/SGS split per chunk so compute can
        # start as soon as its piece lands.
        sb_UH = singles.tile([KF, HALF], fp16)
        nc.sync.dma_start(out=sb_UH, in_=UH.ap())
        sb_VH = singles.tile([KF, P], fp16)
        nc.sync.dma_start(out=sb_VH[:, 0:QCH], in_=VH.ap()[:, 0:QCH])
        sb_MASKS = singles.tile([128, NMASK * MW], fp16)
        nc.sync.dma_start(out=sb_MASKS, in_=MASKS.ap())
        sb_STG = singles.tile([128, NBLK * K2], fp16)
        nc.sync.dma_start(out=sb_STG, in_=STG.ap())
        for cc in range(1, NCH):
            nc.sync.dma_start(out=sb_VH[:, cc * QCH:(cc + 1) * QCH],
                              in_=VH.ap()[:, cc * QCH:(cc + 1) * QCH])
        sb_SGS = singles.tile([K2, P], fp32)
        for cc in range(NCH):
            nc.sync.dma_start(out=sb_SGS[:, cc * QCH:(cc + 1) * QCH],
                              in_=SGS.ap()[:, cc * QCH:(cc + 1) * QCH])

        def body():
            cols = accp.tile([K2, NCH], fp32)
            for c in range(NCH):
                AS = aspool.tile([128, QCH], fp32)
                nblk_c = 4 * c + 4          # blocks participating in chunk c
                for i in range(nblk_c):
                    m = i - 4 * c           # band index (>=0 for band blocks)
                    lo = MW * m if m >= 0 else 0
                    # 512-bank-aligned matmul pieces covering [lo, QCH)
                    pieces = ([(lo, 512 - lo), (512, 512)] if lo < 512
                              else [(lo, QCH - lo)])
                    G = gpool.tile([128, QCH], fp32)
                    Wm = wpool.tile([128, QCH], fp16)
                    for (o, w) in pieces:
                        nc.tensor.matmul(
                            G[:, o:o + w],
                            lhsT=sb_UH[:, i * 128:(i + 1) * 128],
                            rhs=sb_VH[:, c * QCH + o: c * QCH + o + w],
                            start=True,
                            stop=True,
                            skip_group_check=True,
                        )
                    nc.scalar.activation(
                        out=Wm[:, lo:QCH], in_=G[:, lo:QCH],
                        func=mybir.ActivationFunctionType.Exp
                    )
                    if m >= 0:              # diagonal band: elementwise mask
                        nc.vector.tensor_tensor(
                            out=Wm[:, lo:lo + MW],
                            in0=Wm[:, lo:lo + MW],
                            in1=sb_MASKS[:, m * MW:(m + 1) * MW],
                            op=mybir.AluOpType.mult,
                        )
                    for (o, w) in pieces:
                        nc.tensor.matmul(
                            AS[0:K2, o:o + w],
                            lhsT=sb_STG[:, i * K2:(i + 1) * K2],
                            rhs=Wm[:, o:o + w],
                            start=(i == 0),
                            stop=(i == nblk_c - 1),
                            skip_group_check=True,
                        )
                prod = epool.tile([K2, QCH], fp32)
                nc.vector.tensor_tensor(
                    out=prod,
                    in0=AS[0:K2, :],
                    in1=sb_SGS[:, c * QCH:(c + 1) * QCH],
                    op=mybir.AluOpType.mult,
                )
                nc.vector.reduce_sum(
                    out=cols[:, c:c + 1], in_=prod, axis=mybir.AxisListType.X
                )
            acc = accp.tile([K2, 1], fp32)
            nc.vector.reduce_sum(out=acc, in_=cols, axis=mybir.AxisListType.X)
            return acc

        if loop_n == 1:
            acc = body()
        else:
            with tc.For_i(0, loop_n) as _:
                acc = body()
        nc.sync.dma_start(out=OUT.ap(), in_=acc)

    nc.compile()
    return nc


def get_module(loop_n=1):
    key = ("nc", loop_n)
    if key not in _CACHE:
        _CACHE[key] = _build_module(loop_n)
    return _CACHE[key]


def _band_masks(parity):
    """4 mask variants [128, MW] applied to chunk cols [MW*m, MW*m+MW) of band
    block m (global g = 8c + 2m + parity). Within that 256-col window the
    diagonal 128-block sits at offset 128*parity: cols left of it -> 0,
    diag block -> strict-upper + 0.5*diag, right of it -> 1."""
    d = np.triu(np.ones((128, 128), np.float32), 1) + 0.5 * np.eye(128, dtype=np.float32)
    mask = np.zeros((128, MW), np.float32)
    if parity == 0:
        mask[:, 0:128] = d
        mask[:, 128:] = 1.0
    else:
        mask[:, 128:] = d
    out = np.tile(mask.astype(np.float16), (1, NMASK))
    return np.ascontiguousarray(out)


def preprocess(images, segmentations, ROIs, seg_label):
    """Host-side prep: resizes, gating, feature build, per-core sharding."""
    images = np.asarray(images, dtype=np.float32)
    seg = np.asarray(segmentations, dtype=np.float32)
    roi = np.asarray(ROIs, dtype=np.float32)
    lbl = np.asarray(seg_label)

    img_s = images[:, :, ::2, ::2]                    # nearest resize x0.5
    roi_s = roi[:, ::2, ::2]
    lbl_s = lbl[:, :, ::2, ::2]
    seg_s = 0.25 * (seg[:, :, ::2, ::2] + seg[:, :, 1::2, ::2]
                    + seg[:, :, ::2, 1::2] + seg[:, :, 1::2, 1::2])

    unlabel = (lbl_s == 255)[:, 0]
    gate = np.maximum(
        np.where(unlabel, np.float32(1.0), roi_s - seg_s.max(axis=1)), 0.0
    ).astype(np.float32)
    S = (seg_s * roi_s[:, None]).reshape(N_IMG, K, P).astype(np.float32)
    SG = (S * gate.reshape(N_IMG, 1, P)).astype(np.float32)

    yy, xx = np.meshgrid(np.arange(HO, dtype=np.float32),
                         np.arange(WO, dtype=np.float32), indexing="ij")
    pos = np.stack([xx.ravel() / SXY, yy.ravel() / SXY], axis=-1)  # [P,2]

    masks = [_band_masks(0), _band_masks(1)]
    in_maps = []
    for n in range(N_IMG):
        col = img_s[n].reshape(3, P).T / SIGMA_RGB
        f = np.concatenate([pos, col], axis=-1).astype(np.float32)  # [P,5]
        sq = np.sum(f * f, axis=-1)
        ones = np.ones((P, 1), np.float32)
        u = np.concatenate([f, -0.5 * sq[:, None], ones], axis=1)   # [P,7]
        v = np.concatenate([f, ones, -0.5 * sq[:, None]], axis=1)
        uh = u.astype(np.float16)
        ul = (u - uh.astype(np.float32)).astype(np.float16)
        vh = v.astype(np.float16)
        vl = (v - vh.astype(np.float32)).astype(np.float16)
        U = np.concatenate([uh, uh, ul], axis=1)                    # [P,21] fp16
        V = np.concatenate([vh, vl, vh], axis=1)                    # [P,21] fp16
        STf = S[n].T.astype(np.float16)                             # [P,K]
        SGf = SG[n].T.astype(np.float16)                            # [P,K]
        SGS = np.concatenate([SG[n], S[n]], axis=0)                 # [42,P] fp32
        for hh in range(2):
            gsel = np.arange(hh, 32, 2)                 # global blocks, parity hh
            psel = (gsel[:, None] * 128 + np.arange(128)[None, :]).ravel()
            stg = np.concatenate(
                [STf[psel].reshape(NBLK, 128, K),
                 SGf[psel].reshape(NBLK, 128, K)], axis=2)
            stg = stg.transpose(1, 0, 2).reshape(128, NBLK * K2)
            in_maps.append({
                "UH": np.ascontiguousarray(U[psel].T),  # [21, HALF] fp16
                "VH": np.ascontiguousarray(V.T),        # [21, P] fp16
                "STG": np.ascontiguousarray(stg),       # [128, NBLK*42] fp16
                "SGS": SGS,                             # [42, P] fp32
                "MASKS": masks[hh],                     # [128, 4*QCH] fp16
            })
    return in_maps


def kernel(images, segmentations, ROIs, seg_label):
    from concourse.bass_utils import run_bass_kernel_spmd

    nc = get_module()
    in_maps = preprocess(images, segmentations, ROIs, seg_label)
    res = run_bass_kernel_spmd(nc, in_maps, list(range(N_CORES)))
    total = 0.0
    for r in res.results:
        total += float(r["out"].sum())
    return np.array([-WEIGHT * total / N_IMG], dtype=np.float32)

